# revision 5
# baseline (speedup 1.0000x reference)
"""MemNN (end-to-end memory network) Trainium2 kernel.

All the heavy FLOPs are six (B*L, V) @ (V, D) embedding matmuls sharing
`facts` as LHS (A_h = facts @ Wa[h], C_h = facts @ Wc[h]), fused into one
(3200, 10000) @ (10000, 1536) matmul independent of the hop recurrence.

Sharding: vocab (contraction) split 8 ways; each core computes a partial
product, host sums the 8 partials and runs the tiny hop recurrence.

Precision schedule (tolerance is 2e-2 relative): every matmul runs in fp8
e4m3 with MatmulPerfMode.DoubleRow (two 128-row contraction tiles per
instruction, 2x+ PE rate).  fp8's ~2.6% rms quantization noise alone fails
the tolerance, so:
 - facts are mean-shifted (f - 0.5, halving quantization noise) and split
   into hi + lo e4m3 terms (f_hi + f_lo ~ exact); the exact rank-1 shift
   corrections are applied on the host: match += 0.5*(u . colsum(Wa)),
   att += 0.5*colsum(Wc) (p sums to 1).
 - A-half (768 cols), which feeds the softmax-amplified attention logits,
   uses both facts terms: A = (f_hi + f_lo) @ Wa_q, and optionally a third
   term f_hi @ Wa_lo (A_TERMS=3) that makes Wa ~ exact too.
 - C-half (768 cols) only enters through the smooth p-weighted average, so
   a single term f_hi @ Wc_q suffices.
 - Weights are pre-scaled by 2^11 (their ~0.02 entries land in fp8's
   normal range), facts by 2^9; the host divides the partials by 2^20.
 - Question embedding (0.3% of FLOPs): exact fp32r, tail of the kernel.

End-to-end relative error ~1.2e-2 (A_TERMS=2) / ~8.3e-3 (A_TERMS=3),
numpy-simulated on the exact inputs and previously validated to track
hardware within ~2e-5.
"""

import os

os.environ.setdefault("MYCRO_LOCAL_CACHE", "1")

import ml_dtypes
import numpy as np

import concourse.bass as bass
import concourse.mybir as mybir
import concourse.tile as tile
from concourse.bass_utils import run_bass_kernel_spmd

HOPS, B, L, V, D = 3, 64, 50, 10000, 256
NCORES = 8
BL = B * L                # 3200 moving rows
NA = HOPS * D             # 768 A cols: [Wa0|Wa1|Wa2]
NF = 2 * HOPS * D         # 1536 total output rows of pac_t
VSH = V // NCORES         # 1250 vocab rows per core
KT = 10                   # contraction tiles of 128 per core
VPAD = KT * 128           # 1280 (zero-padded)
MCH = 400                 # moving-col chunk
NN = NA // 128            # 6 stationary tiles per half
FSC = 256.0               # 2^8 facts pre-scale for fp8 (e4m3 max finite is 240)
WSC = 2048.0              # 2^11 weight pre-scale for fp8
A_TERMS = 2               # 2: A=(f_hi+f_lo)@Wa_q; 3: + f_hi@Wa_lo
F32R = mybir.dt.float32r
F32 = mybir.dt.float32
FP8 = mybir.dt.float8e4
NP_FP8 = ml_dtypes.float8_e4m3
DR = mybir.MatmulPerfMode.DoubleRow

_nc_cache = None
_last_result = None       # BassKernelResults of the most recent run (for profiling)


def _legalize_sync(nc):
    """Split multi-wait sync_info into standalone single-wait EventSemaphores.

    The walrus build in this environment enforces the raw-bass contract of at
    most ONE SyncWait per instruction ("Too many sync wait commands" in
    setupSyncWait otherwise), while Tile attaches every needed wait to the
    consuming instruction.  Hoisting all-but-one wait onto preceding
    InstEventSemaphore instructions on the same engine queue is semantically
    identical: engine queues are in-order, so a preceding wait blocks the
    queue exactly like an attached wait.  Updates are left untouched (they
    fire at completion and cannot be hoisted).
    """
    for func in nc.m.functions:
        for block in func.blocks:
            insts = list(block.instructions)
            out = []
            n = 0
            for inst in insts:
                si = inst.sync_info
                if si is not None and len(si.on_wait) > 1:
                    waits = list(si.on_wait)
                    for w in waits[:-1]:
                        ev = mybir.InstEventSemaphore(
                            name=f"{inst.name}-hoistw{n}", ins=[], outs=[]
                        )
                        n += 1
                        ev.engine = inst.engine
                        ev.sync_info = mybir.SyncInfo(on_wait=[w], on_update=[])
                        nc.register_instruction(ev)
                        out.append(ev)
                    inst.sync_info = mybir.SyncInfo(
                        on_wait=[waits[-1]], on_update=list(si.on_update)
                    )
                out.append(inst)
            if len(out) != len(insts):
                block.instructions = out
    return nc


_WIDTHS = [MCH] * (BL // MCH)
_STARTS = [sum(_WIDTHS[:i]) for i in range(len(_WIDTHS))]
assert sum(_WIDTHS) == BL


def _build(reps=1):
    """Build the SPMD device program.

    reps>1 repeats the main loop body (same data, same output addresses) --
    used only by the benchmark harness to measure device time differentially
    (per-call dispatch noise over the axon tunnel is ~ms, device time is
    ~100 us, so wall-clocking one launch cannot resolve it).
    """
    nc = bass.Bass(trn_type="TRN2")
    f_hi = nc.dram_tensor("f_hi", [VPAD, BL], FP8, kind="ExternalInput")
    f_lo = nc.dram_tensor("f_lo", [VPAD, BL], FP8, kind="ExternalInput")
    wa_8 = nc.dram_tensor("wa_8", [VPAD, NA], FP8, kind="ExternalInput")
    wc_8 = nc.dram_tensor("wc_8", [VPAD, NA], FP8, kind="ExternalInput")
    if A_TERMS == 3:
        wal_8 = nc.dram_tensor("wal_8", [VPAD, NA], FP8, kind="ExternalInput")
    q_t = nc.dram_tensor("q_t", [VPAD, B], F32R, kind="ExternalInput")
    wq = nc.dram_tensor("wq", [VPAD, D], F32R, kind="ExternalInput")
    pac_t = nc.dram_tensor("pac_t", [NF, BL], F32, kind="ExternalOutput")
    pu = nc.dram_tensor("pu", [B, D], F32, kind="ExternalOutput")

    fhr = f_hi.rearrange("(k p) n -> p k n", p=128)
    flr = f_lo.rearrange("(k p) n -> p k n", p=128)
    war = wa_8.rearrange("(k p) n -> p k n", p=128)
    wcr = wc_8.rearrange("(k p) n -> p k n", p=128)
    if A_TERMS == 3:
        walr = wal_8.rearrange("(k p) n -> p k n", p=128)
    qr = q_t.rearrange("(k p) n -> p k n", p=128)
    wqr = wq.rearrange("(k p) n -> p k n", p=128)
    wmax = max(_WIDTHS)

    with (
        tile.TileContext(nc) as tc,
        tc.tile_pool(name="wpool", bufs=1) as wpool,
        tc.tile_pool(name="xhpool", bufs=3) as xhpool,
        tc.tile_pool(name="xlpool", bufs=3) as xlpool,
        tc.tile_pool(name="opool", bufs=6) as opool,
        tc.tile_pool(name="pspool", bufs=7, space="PSUM") as pspool,
    ):
        # Prologue DMA order: first wa_8 n-slice 0 + first f_hi chunk (the
        # first matmul group's deps), then the remaining weights, then the
        # small question tensors.
        wat = wpool.tile([128, KT, NA], FP8)
        nc.sync.dma_start(wat[:, :, 0:128], war[:, :, 0:128])
        xhs = {}
        xls = {}
        xhs[0] = xhpool.tile(
            [128, KT, _WIDTHS[0]], FP8, tag="xh", name="xh",
            padded_shape=[128, KT, wmax],
        )
        nc.sync.dma_start(xhs[0][:], fhr[:, :, 0 : _WIDTHS[0]])
        for off in range(128, NA, 384):
            end = min(off + 384, NA)
            nc.sync.dma_start(wat[:, :, off:end], war[:, :, off:end])
        xls[0] = xlpool.tile(
            [128, KT, _WIDTHS[0]], FP8, tag="xl", name="xl",
            padded_shape=[128, KT, wmax],
        )
        nc.sync.dma_start(xls[0][:], flr[:, :, 0 : _WIDTHS[0]])
        if A_TERMS == 3:
            walt = wpool.tile([128, KT, NA], FP8)
            for off in range(0, NA, 384):
                end = min(off + 384, NA)
                nc.sync.dma_start(walt[:, :, off:end], walr[:, :, off:end])
        wct = wpool.tile([128, KT, NA], FP8)
        for off in range(0, NA, 384):
            end = min(off + 384, NA)
            nc.sync.dma_start(wct[:, :, off:end], wcr[:, :, off:end])
        qtile = wpool.tile([128, KT, B], F32R)
        nc.sync.dma_start(qtile[:], qr)
        wqt = wpool.tile([128, KT, D], F32R)
        nc.sync.dma_start(wqt[:], wqr)

        def get_xt(mi, xs, pool, rr, tg):
            if mi not in xs:
                xs[mi] = pool.tile(
                    [128, KT, _WIDTHS[mi]], FP8, tag=tg, name=tg,
                    padded_shape=[128, KT, wmax],
                )
                nc.sync.dma_start(
                    xs[mi][:], rr[:, :, _STARTS[mi] : _STARTS[mi] + _WIDTHS[mi]]
                )
            return xs[mi]

        def drain(ps, n, mi):
            ot = opool.tile(
                [128, _WIDTHS[mi]], F32, tag="ot", name="ot",
                padded_shape=[128, wmax],
            )
            nc.vector.tensor_copy(ot[:], ps[:])
            nc.sync.dma_start(
                pac_t[
                    n * 128 : (n + 1) * 128,
                    _STARTS[mi] : _STARTS[mi] + _WIDTHS[mi],
                ],
                ot[:],
            )

        # Main fused matmul, all fp8 DoubleRow (2 k-tiles per instruction).
        # A-half accumulates hi+lo facts terms (and optionally the Wa_lo
        # term) into one PSUM group; C-half is the single hi term.
        for _ in range(reps):
            for mi in range(len(_WIDTHS)):
                xh = get_xt(mi, xhs, xhpool, fhr, "xh")
                xl = get_xt(mi, xls, xlpool, flr, "xl")
                for n in range(NN):
                    ps = pspool.tile(
                        [128, _WIDTHS[mi]], F32, tag="ps", name="ps",
                        padded_shape=[128, wmax],
                    )
                    groups = [(wat, xh), (wat, xl)]
                    if A_TERMS == 3:
                        groups.append((walt, xh))
                    last = len(groups) * (KT // 2) - 1
                    i = 0
                    for wt_, xt_ in groups:
                        for t in range(KT // 2):
                            nc.tensor.matmul(
                                ps[:],
                                wt_[:, 2 * t : 2 * t + 2, n * 128 : (n + 1) * 128],
                                xt_[:, 2 * t : 2 * t + 2, :],
                                start=(i == 0),
                                stop=(i == last),
                                perf_mode=DR,
                            )
                            i += 1
                    drain(ps, n, mi)
                for n in range(NN):
                    ps = pspool.tile(
                        [128, _WIDTHS[mi]], F32, tag="ps", name="ps",
                        padded_shape=[128, wmax],
                    )
                    for t in range(KT // 2):
                        nc.tensor.matmul(
                            ps[:],
                            wct[:, 2 * t : 2 * t + 2, n * 128 : (n + 1) * 128],
                            xh[:, 2 * t : 2 * t + 2, :],
                            start=(t == 0),
                            stop=(t == KT // 2 - 1),
                            perf_mode=DR,
                        )
                    drain(ps, NN + n, mi)
            xhs.clear()
            xls.clear()

        # Question embedding at the tail: its PE work (10 small matmuls)
        # overlaps the main loop's epilogue.
        psq = pspool.tile([B, D], F32, tag="psq", bufs=1)
        for k in range(KT):
            nc.tensor.matmul(
                psq[:], qtile[:, k, :], wqt[:, k, :], start=(k == 0), stop=(k == KT - 1)
            )
        uo = opool.tile([B, D], F32, tag="uo")
        nc.any.tensor_copy(out=uo[:], in_=psq[:])
        nc.sync.dma_start(pu[:, :], uo[:])
    return _legalize_sync(nc)


def _shard_inputs(facts, question, Wq, Wa, Wc):
    fx = np.ascontiguousarray(facts, dtype=np.float32).reshape(BL, V)
    fs = (fx - np.float32(0.5)) * np.float32(FSC)
    fhi = fs.astype(NP_FP8)
    flo = (fs - fhi.astype(np.float32)).astype(NP_FP8)
    qx = np.asarray(question, dtype=np.float32).sum(axis=1)  # (B, V) bag-of-words
    Wq = np.asarray(Wq, dtype=np.float32)
    Wa = np.asarray(Wa, dtype=np.float32)
    Wc = np.asarray(Wc, dtype=np.float32)
    was = np.concatenate([Wa[0], Wa[1], Wa[2]], axis=1) * np.float32(WSC)
    wa8 = was.astype(NP_FP8)
    wc8 = (
        np.concatenate([Wc[0], Wc[1], Wc[2]], axis=1) * np.float32(WSC)
    ).astype(NP_FP8)
    if A_TERMS == 3:
        wal8 = (was - wa8.astype(np.float32)).astype(NP_FP8)

    in_maps = []
    for c in range(NCORES):
        sl = slice(c * VSH, (c + 1) * VSH)
        fh = np.zeros((VPAD, BL), NP_FP8)
        fh[:VSH] = fhi[:, sl].T
        fl = np.zeros((VPAD, BL), NP_FP8)
        fl[:VSH] = flo[:, sl].T
        qt = np.zeros((VPAD, B), np.float32)
        qt[:VSH] = qx[:, sl].T
        wab = np.zeros((VPAD, NA), NP_FP8)
        wab[:VSH] = wa8[sl]
        wcb = np.zeros((VPAD, NA), NP_FP8)
        wcb[:VSH] = wc8[sl]
        wqs = np.zeros((VPAD, D), np.float32)
        wqs[:VSH] = Wq[sl]
        m = {"f_hi": fh, "f_lo": fl, "q_t": qt, "wa_8": wab,
             "wc_8": wcb, "wq": wqs}
        if A_TERMS == 3:
            wlb = np.zeros((VPAD, NA), NP_FP8)
            wlb[:VSH] = wal8[sl]
            m["wal_8"] = wlb
        in_maps.append(m)
    return in_maps


def _wait_for_devices(min_wait_attempts=10):
    """The axon terminal occasionally reports a transient bad topology
    ("terminal has 1 core"); poll until all 8 NeuronCores are visible."""
    import time as _time

    import jax

    for attempt in range(min_wait_attempts):
        try:
            if len(jax.devices()) >= NCORES:
                return
        except Exception:  # noqa: BLE001 - backend init failure is retryable
            try:
                jax.clear_backends()
            except Exception:  # noqa: BLE001
                pass
        _time.sleep(15.0)
    # fall through: let the run itself raise a descriptive error


def _run_with_retries(nc, in_maps, attempts=4):
    """run_bass_kernel_spmd with retries: the axon terminal occasionally
    reports transient failures (device wedged / NRT_EXEC_UNIT_UNRECOVERABLE /
    temporary topology glitches) that succeed on re-dispatch."""
    import time as _time

    last_exc = None
    for attempt in range(attempts):
        try:
            return run_bass_kernel_spmd(nc, in_maps, list(range(NCORES)))
        except Exception as e:  # noqa: BLE001 - retry any runtime failure
            last_exc = e
            if attempt < attempts - 1:
                _time.sleep(10.0 * (attempt + 1))
                _wait_for_devices(min_wait_attempts=4)
    raise last_exc


def kernel(facts, question, Wq, Wa, Wc, Ww, bw):
    global _nc_cache, _last_result
    _wait_for_devices(min_wait_attempts=8)
    in_maps = _shard_inputs(facts, question, Wq, Wa, Wc)
    if _nc_cache is None:
        _nc_cache = _build()
    _last_result = _run_with_retries(_nc_cache, in_maps)
    res = _last_result.results

    # Unshard: sum the 8 partial products of the vocab-sharded matmul.
    ac_t = res[0]["pac_t"].copy()
    u = res[0]["pu"].copy()
    for r in res[1:]:
        ac_t += r["pac_t"]
        u += r["pu"]
    ac_t *= np.float32(1.0 / (FSC * WSC))  # undo fp8 pre-scales

    Wa = np.asarray(Wa, dtype=np.float32)
    Wc = np.asarray(Wc, dtype=np.float32)
    colsum_wa = Wa.sum(axis=1)  # (HOPS, D): exact rank-1 shift corrections
    colsum_wc = Wc.sum(axis=1)

    # Sequential hop recurrence (tiny: ~30 MFLOP vs 98.3 GFLOP on device).
    Ww = np.asarray(Ww, dtype=np.float32)
    bw = np.asarray(bw, dtype=np.float32)
    for h in range(HOPS):
        A = ac_t[h * D : (h + 1) * D].reshape(D, B, L)
        C = ac_t[(HOPS + h) * D : (HOPS + h + 1) * D].reshape(D, B, L)
        match = np.einsum("dbl,bd->bl", A, u)
        # facts were mean-shifted by 0.5 before fp8 quantization; these are
        # the exact rank-1 corrections (p sums to 1 for the att one).
        match += np.float32(0.5) * (u @ colsum_wa[h])[:, None]
        mm = match - match.max(axis=-1, keepdims=True)
        e = np.exp(mm)
        p = e / e.sum(axis=-1, keepdims=True)
        att = np.einsum("bl,dbl->bd", p, C)
        att += np.float32(0.5) * colsum_wc[h]
        z = (u + att) @ Ww[h] + bw[h]
        if h == HOPS - 1:
            zz = z - z.max(axis=-1, keepdims=True)
            ez = np.exp(zz)
            u = ez / ez.sum(axis=-1, keepdims=True)
        else:
            u = np.maximum(z, 0.0)
    return np.ascontiguousarray(u, dtype=np.float32)
